# revision 19
# baseline (speedup 1.0000x reference)
"""Trainium2 Bass kernel: MHSA with multi-head relative position embedding.

Sharding: data-parallel over batch - 16 batches / 8 cores = 2 batches per core,
each core computes all 8 heads for its 2 batches. No collectives needed.

Math per batch (N=784 tokens, C=512, 8 heads x 64 dim):
  qkv = x @ w_qkv                  (q-columns pre-scaled by 1/8 on host)
  scoresT[k,q] = logbiasT[h,k,q] + k_h^T q_h   (bias Toeplitz-gathered on host
                                   in LOG domain, added into PSUM by an
                                   identity matmul, scores accumulate on top)
  E = exp(scoresT)                 (one ACT instr per [112,784] tile)
  O_T[d,q] = sum_k v_aug[k,d] E[k,q]  with v_aug = [v | 1] -> row 64 = sumexp
  attnT = O_T[0:64] / O_T[64]
  out = attn^T stacked over heads @ w_out

Pipelining: per head-pair, two streams (even/odd head); the id-matmul ->
scores -> exp chain of one stream overlaps the other stream's work and the
trailing attn@v matmuls. PSUM: 4 banks of scores (2 streams x [112,784]) +
4 banks of O accumulators. VMM_SPLIT (pairing split-K attn@v halves on
disjoint PE row-tiles) is OFF: concurrent accumulating (start=False) matmul
drains crash real TRN2 hardware in every tested arrangement, though CoreSim
accepts them; only start=stop=True concurrent pairs (the score matmuls) are
hardware-proven.
"""

import numpy as np
import ml_dtypes

IDMM_BIAS = True    # bias via identity-matmul into PSUM (else DVE multiply)

B, HH, WW, C = 16, 28, 28, 512
N = HH * WW            # 784 tokens
HEADS, KD = 8, 64
NCORES, BPC = 8, 2     # 8 cores, 2 batches per core
NT, TP = 7, 112        # 784 = 7 tiles of 112 (k / token tiling)
CHUNKS = [(0, 512), (512, 272)]   # q-chunks for qkv/proj (bank = 512 fp32)
PASSES = [(0, 392), (392, 392)]   # q-passes for attention (1 bank per pass)
CT = 4                 # contraction tiles of 128 over C=512

_CACHE = {}


def _rel_index():
    # Faithful to reference._relative_position_index: token r -> (r%28, r//28)
    t = np.arange(N)
    c0, c1 = t % HH, t // HH
    return ((c0[:, None] - c0[None, :] + HH - 1)
            + (c1[:, None] - c1[None, :] + WW - 1) * (2 * HH - 1))  # [q, k]


def build_nc():
    if 'nc' in _CACHE:
        return _CACHE['nc']
    from contextlib import ExitStack
    import concourse.bacc as bacc
    import concourse.mybir as mybir
    import concourse.tile as tile
    from concourse.alu_op_type import AluOpType

    f32 = mybir.dt.float32
    bf16 = mybir.dt.bfloat16
    EXP = mybir.ActivationFunctionType.Exp

    nc = bacc.Bacc("TRN2", debug=False, enable_asserts=False)
    xT_d = nc.dram_tensor("xT", [BPC, C, N], bf16, kind="ExternalInput").ap()
    wqkv_d = nc.dram_tensor("wqkv", [C, 3 * C], bf16, kind="ExternalInput").ap()
    wout_d = nc.dram_tensor("wout", [C, C], bf16, kind="ExternalInput").ap()
    bias_d = nc.dram_tensor("biasT", [HEADS, N, N], bf16, kind="ExternalInput").ap()
    ident_d = nc.dram_tensor("ident", [128, 128], bf16, kind="ExternalInput").ap()
    out_d = nc.dram_tensor("out", [BPC, N, C], f32, kind="ExternalOutput").ap()

    with tile.TileContext(nc) as tc, ExitStack() as ctx:
        persist = ctx.enter_context(tc.tile_pool(name="persist", bufs=1))
        xT_pool = ctx.enter_context(tc.tile_pool(name="xTp", bufs=8))
        bias_pool = ctx.enter_context(tc.tile_pool(name="biasp", bufs=28))
        e_pool = ctx.enter_context(tc.tile_pool(name="ep", bufs=12))
        r_pool = ctx.enter_context(tc.tile_pool(name="rp", bufs=3))
        osb_pool = ctx.enter_context(tc.tile_pool(name="osbp", bufs=2))
        sc_psum = ctx.enter_context(tc.tile_pool(name="scp", bufs=4, space="PSUM"))
        o_psum = ctx.enter_context(tc.tile_pool(name="opp", bufs=4, space="PSUM"))
        pj_psum = o_psum  # share banks: proj/qkv phases don't overlap attention O

        # ---- weights + identity resident in SBUF ----
        wqkv_sb, wout_sb = [], []
        for ci in range(CT):
            w = persist.tile([128, 3 * C], bf16, tag=f"wqkv{ci}", name=f"wqkv{ci}")
            nc.sync.dma_start(w, wqkv_d[ci * 128:(ci + 1) * 128, :])
            wqkv_sb.append(w)
        for ci in range(CT):
            w = persist.tile([128, C], bf16, tag=f"wout{ci}", name=f"wout{ci}")
            nc.sync.dma_start(w, wout_d[ci * 128:(ci + 1) * 128, :])
            wout_sb.append(w)
        ident_sb = persist.tile([128, 128], bf16, tag="ident", name="ident")
        nc.sync.dma_start(ident_sb, ident_d)

        qkT, vsb, attnT = {}, {}, {}
        for b in range(BPC):
            for fi in range(CT):
                attnT[b, fi] = persist.tile(
                    [128, N], bf16, tag=f"attnT{b}_{fi}", name=f"attnT{b}_{fi}")

        # ---- qkv projection, split so head-pair 0 can start early ----
        def emit_xt(b):
            tiles = []
            for ci in range(CT):
                xt = xT_pool.tile([128, N], bf16, tag="xT", name=f"xT{b}_{ci}")
                nc.sync.dma_start(xt, xT_d[b, ci * 128:(ci + 1) * 128, :])
                tiles.append(xt)
            return tiles

        def emit_qk_tile(b, ft, xT_sb):
            dst = persist.tile([128, N], bf16, tag=f"qkT{b}_{ft}",
                               name=f"qkT{b}_{ft}")
            qkT[b, ft] = dst
            for (c0w, cw) in CHUNKS:
                ps = pj_psum.tile([128, 512], f32, tag="op", name=f"pj{b}_{ft}_{c0w}")
                for ci in range(CT):
                    nc.tensor.matmul(
                        ps[:, 0:cw], wqkv_sb[ci][:, ft * 128:(ft + 1) * 128],
                        xT_sb[ci][:, c0w:c0w + cw],
                        start=(ci == 0), stop=(ci == CT - 1))
                nc.vector.tensor_copy(dst[:, c0w:c0w + cw], ps[:, 0:cw])

        def emit_v(b, xT_sb):
            for t in range(NT):
                vt = persist.tile([TP, HEADS, KD + 1], bf16, tag=f"v{b}_{t}",
                                  name=f"v{b}_{t}")
                vsb[b, t] = vt
                ps = pj_psum.tile([TP, 512], f32, tag="op", name=f"pv{b}_{t}")
                for ci in range(CT):
                    nc.tensor.matmul(
                        ps, xT_sb[ci][:, t * TP:(t + 1) * TP],
                        wqkv_sb[ci][:, 2 * C:3 * C],
                        start=(ci == 0), stop=(ci == CT - 1))
                nc.vector.tensor_copy(
                    vt[:, :, 0:KD], ps.rearrange("p (h d) -> p h d", h=HEADS))
                nc.vector.memset(vt[:, :, KD:KD + 1], 1.0)

        # ---- attention for one head pair ----
        def attention_pair(hp):
            h0, h1 = 2 * hp, 2 * hp + 1
            with nc.named_scope(f"headpair{hp}"):
                bias_sb = {}
                for h in (h0, h1):
                    for kt in range(NT):
                        bt = bias_pool.tile([TP, N], bf16, tag="bias",
                                            name=f"bias{h}_{kt}")
                        nc.sync.dma_start(bt, bias_d[h, kt * TP:(kt + 1) * TP, :])
                        bias_sb[h, kt] = bt
                for b in range(BPC):
                    attention_group(hp, b, bias_sb)

        def attention_group(hp, b, bias_sb):
            h0, h1 = 2 * hp, 2 * hp + 1
            # Scores psum: per (stream, pass) one bank, ring of 4 -> true
            # double buffering across kt (no exp-wait stall on the PE).
            # Emission per kt: paired score matmuls FIRST (start=True: the
            # hardware-proven 0x1-flag concurrency class, disjoint PE row
            # halves 0-63 / 64-127), then full-array bias id-matmuls that
            # accumulate on top and close the group, then exp, then the
            # trailing attn@v matmuls of the previous tile.
            ovt = {}
            nv = {}
            for h in (h0, h1):
                for pi, (c0w, cw) in enumerate(PASSES):
                    ovt[h, pi] = o_psum.tile([KD + 1, 392], f32, tag="op",
                                             name=f"o{h}_{b}_{pi}")
                    nv[h, pi] = 0

            esb = {}

            def vmm(h, pi, kt):
                nc.tensor.matmul(
                    ovt[h, pi], vsb[b, kt][:, h, :], esb[h, kt, pi],
                    start=(nv[h, pi] == 0), stop=(nv[h, pi] == NT - 1))
                nv[h, pi] += 1

            for kt in range(NT):
                scp = {}
                for pi, (c0w, cw) in enumerate(PASSES):
                    for h in (h0, h1):
                        r0 = (h % 2) * 64
                        scp[h, pi] = sc_psum.tile(
                            [TP, 392], f32, tag="sc", name=f"sc{h}_{b}_{kt}_{pi}")
                        nc.tensor.matmul(
                            scp[h, pi],
                            qkT[b, 4 + h // 2][r0:r0 + 64,
                                               kt * TP:(kt + 1) * TP],
                            qkT[b, h // 2][r0:r0 + 64, c0w:c0w + cw],
                            start=True, stop=False)
                for h in (h0, h1):
                    for pi, (c0w, cw) in enumerate(PASSES):
                        nc.tensor.matmul(
                            scp[h, pi], ident_sb[0:TP, 0:TP],
                            bias_sb[h, kt][:, c0w:c0w + cw],
                            start=False, stop=True)
                    for pi in range(len(PASSES)):
                        et = e_pool.tile([TP, 392], bf16, tag="e",
                                         name=f"e{h}_{b}_{kt}_{pi}")
                        nc.scalar.activation(et, scp[h, pi], EXP)
                        esb[h, kt, pi] = et
                if kt > 0:
                    for h in (h0, h1):
                        for pi in range(len(PASSES)):
                            vmm(h, pi, kt - 1)
            for h in (h0, h1):
                for pi in range(len(PASSES)):
                    vmm(h, pi, NT - 1)
            # normalize: rows 0..63 * (1 / row 64); recip must not read PSUM
            for h in (h0, h1):
                r0 = (h % 2) * 64
                for pi, (c0w, cw) in enumerate(PASSES):
                    srow = r_pool.tile([1, 512], f32, tag="srow", name="srow")
                    nc.vector.tensor_copy(
                        srow[:, 0:cw], ovt[h, pi][KD:KD + 1, 0:cw])
                    rrow = r_pool.tile([1, 512], f32, tag="rrow", name="rrow")
                    nc.vector.reciprocal_approx_fast(
                        rrow[:, 0:cw], srow[:, 0:cw])
                    rb = r_pool.tile([64, 512], f32, tag="rb", name="rb")
                    nc.gpsimd.partition_broadcast(
                        rb[:, 0:cw], rrow[:, 0:cw])
                    nc.vector.tensor_tensor(
                        attnT[b, h // 2][r0:r0 + 64, c0w:c0w + cw],
                        ovt[h, pi][0:KD, 0:cw], rb[:, 0:cw],
                        AluOpType.mult)

        # phase 1: minimal inputs for head-pair 0 (q-tile 0, k-tile 4, v)
        xts = {}
        for b in range(BPC):
            with nc.named_scope(f"qkv_early_b{b}"):
                xts[b] = emit_xt(b)
                emit_qk_tile(b, 0, xts[b])
                emit_qk_tile(b, 4, xts[b])
        for b in range(BPC):
            with nc.named_scope(f"v_b{b}"):
                emit_v(b, xts[b])
        # phase 2: pair-0 attention starts while the rest of qkv is emitted
        attention_pair(0)
        # phase 3: remaining q/k feature tiles
        for b in range(BPC):
            with nc.named_scope(f"qkv_rest_b{b}"):
                for ft in (1, 5, 2, 6, 3, 7):
                    emit_qk_tile(b, ft, xts[b])
        # phase 4: remaining head pairs
        for hp in range(1, HEADS // 2):
            attention_pair(hp)

        # ---- output projection ----
        for b in range(BPC):
            with nc.named_scope(f"proj_b{b}"):
                for t in range(NT):
                    ps = pj_psum.tile([TP, 512], f32, tag="op", name=f"po{b}_{t}")
                    for fi in range(CT):
                        nc.tensor.matmul(
                            ps, attnT[b, fi][:, t * TP:(t + 1) * TP], wout_sb[fi],
                            start=(fi == 0), stop=(fi == CT - 1))
                    osb = osb_pool.tile([TP, C], f32, tag="osb", name="osb")
                    nc.vector.tensor_copy(osb, ps)
                    nc.sync.dma_start(out_d[b, t * TP:(t + 1) * TP, :], osb)

    nc.compile()
    _CACHE['nc'] = nc
    return nc


def host_prep(x, w_qkv, pos_table, w_out):
    x = np.asarray(x, np.float32).reshape(B, N, C)
    wq = np.array(np.asarray(w_qkv, np.float32), copy=True)
    wq[:, :C] *= np.float32(1.0 / np.sqrt(KD))
    wq_bf = wq.astype(ml_dtypes.bfloat16)
    idx = _rel_index()
    # bias transposed to [h, k, q]: log-domain (added to scores pre-exp) when
    # IDMM_BIAS, exp-domain (multiplies exp(scores)) otherwise
    bT = np.asarray(pos_table, np.float32)[:, idx].transpose(0, 2, 1)
    if not IDMM_BIAS:
        bT = np.exp(bT)
    biasT = np.ascontiguousarray(bT).astype(ml_dtypes.bfloat16)
    wout = np.ascontiguousarray(np.asarray(w_out, np.float32)).astype(
        ml_dtypes.bfloat16)
    ident = np.eye(128, dtype=ml_dtypes.bfloat16)
    in_maps = []
    for c in range(NCORES):
        xT = np.ascontiguousarray(
            x[c * BPC:(c + 1) * BPC].transpose(0, 2, 1)).astype(
                ml_dtypes.bfloat16)  # [2, 512, 784]
        in_maps.append({"xT": xT, "wqkv": wq_bf, "wout": wout,
                        "biasT": biasT, "ident": ident})
    return in_maps


def run(in_maps, trace=False, trace_cores=None):
    import concourse.bass_utils as bass_utils
    nc = build_nc()
    return bass_utils.run_bass_kernel_spmd(
        nc, in_maps, core_ids=list(range(NCORES)),
        trace=trace, trace_cores=trace_cores)


def kernel(x, w_qkv, pos_table, w_out):
    in_maps = host_prep(x, w_qkv, pos_table, w_out)
    res = run(in_maps)
    out = np.stack([r["out"] for r in res.results])    # [8, 2, 784, 512]
    return np.ascontiguousarray(out.reshape(B, HH, WW, C)).astype(np.float32)


# revision 21
# speedup vs baseline: 1.0856x; 1.0856x over previous
"""Trainium2 Bass kernel: MHSA with multi-head relative position embedding.

Sharding: data-parallel over batch - 16 batches / 8 cores = 2 batches per core,
each core computes all 8 heads for its 2 batches. No collectives needed.

Math per batch (N=784 tokens, C=512, 8 heads x 64 dim):
  qkv = x @ w_qkv                  (q-columns pre-scaled by 1/8 on host)
  scoresT[k,q] = logbiasT[h,k,q] + k_h^T q_h   (bias Toeplitz-gathered on host
                                   in LOG domain, added into PSUM by an
                                   identity matmul, scores accumulate on top)
  E = exp(scoresT)                 (one ACT instr per [112,784] tile)
  O_T[d,q] = sum_k v_aug[k,d] E[k,q]  with v_aug = [v | 1] -> row 64 = sumexp
  attnT = O_T[0:64] / O_T[64]
  out = attn^T stacked over heads @ w_out

Pipelining: per head-pair, two streams (even/odd head); the id-matmul ->
scores -> exp chain of one stream overlaps the other stream's work and the
trailing attn@v matmuls. PSUM: 4 banks of scores (2 streams x [112,784]) +
4 banks of O accumulators. VMM_SPLIT (pairing split-K attn@v halves on
disjoint PE row-tiles) is OFF: concurrent accumulating (start=False) matmul
drains crash real TRN2 hardware in every tested arrangement, though CoreSim
accepts them; only start=stop=True concurrent pairs (the score matmuls) are
hardware-proven.
"""

import numpy as np
import ml_dtypes

IDMM_BIAS = True    # bias via identity-matmul into PSUM (else DVE multiply)

B, HH, WW, C = 16, 28, 28, 512
N = HH * WW            # 784 tokens
HEADS, KD = 8, 64
NCORES, BPC = 8, 2     # 8 cores, 2 batches per core
NT, TP = 7, 112        # 784 = 7 tiles of 112 (k / token tiling)
CHUNKS = [(0, 512), (512, 272)]   # q-chunks for qkv/proj (bank = 512 fp32)
PASSES = [(0, 392), (392, 392)]   # q-passes for attention (1 bank per pass)
CT = 4                 # contraction tiles of 128 over C=512

_CACHE = {}


def _rel_index():
    # Faithful to reference._relative_position_index: token r -> (r%28, r//28)
    t = np.arange(N)
    c0, c1 = t % HH, t // HH
    return ((c0[:, None] - c0[None, :] + HH - 1)
            + (c1[:, None] - c1[None, :] + WW - 1) * (2 * HH - 1))  # [q, k]


def build_nc():
    if 'nc' in _CACHE:
        return _CACHE['nc']
    from contextlib import ExitStack
    import concourse.bacc as bacc
    import concourse.mybir as mybir
    import concourse.tile as tile
    from concourse.alu_op_type import AluOpType

    f32 = mybir.dt.float32
    bf16 = mybir.dt.bfloat16
    EXP = mybir.ActivationFunctionType.Exp

    nc = bacc.Bacc("TRN2", debug=False, enable_asserts=False)
    xT_d = nc.dram_tensor("xT", [BPC, C, N], bf16, kind="ExternalInput").ap()
    wqkv_d = nc.dram_tensor("wqkv", [C, 3 * C], bf16, kind="ExternalInput").ap()
    wout_d = nc.dram_tensor("wout", [C, C], bf16, kind="ExternalInput").ap()
    bias_d = nc.dram_tensor("biasT", [HEADS, N, N], bf16, kind="ExternalInput").ap()
    ident_d = nc.dram_tensor("ident", [128, 128], bf16, kind="ExternalInput").ap()
    out_d = nc.dram_tensor("out", [BPC, N, C], f32, kind="ExternalOutput").ap()

    with tile.TileContext(nc) as tc, ExitStack() as ctx:
        persist = ctx.enter_context(tc.tile_pool(name="persist", bufs=1))
        xT_pool = ctx.enter_context(tc.tile_pool(name="xTp", bufs=8))
        bias_pool = ctx.enter_context(tc.tile_pool(name="biasp", bufs=28))
        e_pool = ctx.enter_context(tc.tile_pool(name="ep", bufs=12))
        r_pool = ctx.enter_context(tc.tile_pool(name="rp", bufs=3))
        osb_pool = ctx.enter_context(tc.tile_pool(name="osbp", bufs=2))
        sc_psum = ctx.enter_context(tc.tile_pool(name="scp", bufs=2, space="PSUM"))
        o_psum = ctx.enter_context(tc.tile_pool(name="opp", bufs=4, space="PSUM"))
        pj_psum = o_psum  # share banks: proj/qkv phases don't overlap attention O

        # ---- weights + identity resident in SBUF ----
        wqkv_sb, wout_sb = [], []
        for ci in range(CT):
            w = persist.tile([128, 3 * C], bf16, tag=f"wqkv{ci}", name=f"wqkv{ci}")
            nc.sync.dma_start(w, wqkv_d[ci * 128:(ci + 1) * 128, :])
            wqkv_sb.append(w)
        for ci in range(CT):
            w = persist.tile([128, C], bf16, tag=f"wout{ci}", name=f"wout{ci}")
            nc.sync.dma_start(w, wout_d[ci * 128:(ci + 1) * 128, :])
            wout_sb.append(w)
        ident_sb = persist.tile([128, 128], bf16, tag="ident", name="ident")
        nc.sync.dma_start(ident_sb, ident_d)

        qkT, vsb, attnT = {}, {}, {}
        for b in range(BPC):
            for fi in range(CT):
                attnT[b, fi] = persist.tile(
                    [128, N], bf16, tag=f"attnT{b}_{fi}", name=f"attnT{b}_{fi}")

        # ---- qkv projection, split so head-pair 0 can start early ----
        def emit_xt(b):
            tiles = []
            for ci in range(CT):
                xt = xT_pool.tile([128, N], bf16, tag="xT", name=f"xT{b}_{ci}")
                nc.sync.dma_start(xt, xT_d[b, ci * 128:(ci + 1) * 128, :])
                tiles.append(xt)
            return tiles

        def emit_qk_tile(b, ft, xT_sb):
            dst = persist.tile([128, N], bf16, tag=f"qkT{b}_{ft}",
                               name=f"qkT{b}_{ft}")
            qkT[b, ft] = dst
            for (c0w, cw) in CHUNKS:
                ps = pj_psum.tile([128, 512], f32, tag="op", name=f"pj{b}_{ft}_{c0w}")
                for ci in range(CT):
                    nc.tensor.matmul(
                        ps[:, 0:cw], wqkv_sb[ci][:, ft * 128:(ft + 1) * 128],
                        xT_sb[ci][:, c0w:c0w + cw],
                        start=(ci == 0), stop=(ci == CT - 1))
                nc.vector.tensor_copy(dst[:, c0w:c0w + cw], ps[:, 0:cw])

        def emit_v(b, xT_sb):
            for t in range(NT):
                vt = persist.tile([TP, HEADS, KD + 1], bf16, tag=f"v{b}_{t}",
                                  name=f"v{b}_{t}")
                vsb[b, t] = vt
                ps = pj_psum.tile([TP, 512], f32, tag="op", name=f"pv{b}_{t}")
                for ci in range(CT):
                    nc.tensor.matmul(
                        ps, xT_sb[ci][:, t * TP:(t + 1) * TP],
                        wqkv_sb[ci][:, 2 * C:3 * C],
                        start=(ci == 0), stop=(ci == CT - 1))
                nc.vector.tensor_copy(
                    vt[:, :, 0:KD], ps.rearrange("p (h d) -> p h d", h=HEADS))
                nc.vector.memset(vt[:, :, KD:KD + 1], 1.0)

        # ---- attention for one head pair ----
        def bias_dma(hp):
            h0, h1 = 2 * hp, 2 * hp + 1
            bias_sb = {}
            for h in (h0, h1):
                for kt in range(NT):
                    bt = bias_pool.tile([TP, N], bf16, tag="bias",
                                        name=f"bias{h}_{kt}")
                    nc.sync.dma_start(bt, bias_d[h, kt * TP:(kt + 1) * TP, :])
                    bias_sb[h, kt] = bt
            return bias_sb

        def attention_group(hp, b, bias_sb):
            h0, h1 = 2 * hp, 2 * hp + 1
            # Scores psum: per (stream, pass) one bank, ring of 4 -> true
            # double buffering across kt (no exp-wait stall on the PE).
            # Emission per kt: paired score matmuls FIRST (start=True: the
            # hardware-proven 0x1-flag concurrency class, disjoint PE row
            # halves 0-63 / 64-127), then full-array bias id-matmuls that
            # accumulate on top and close the group, then exp, then the
            # trailing attn@v matmuls of the previous tile.
            ovt = {}
            nv = {}
            for h in (h0, h1):
                for ci, (c0w, cw) in enumerate(CHUNKS):
                    ovt[h, ci] = o_psum.tile([KD + 1, 512], f32, tag="op",
                                             name=f"o{h}_{b}_{ci}")
                    nv[h, ci] = 0

            esb = {}

            def vmm(h, ci, kt):
                (c0w, cw) = CHUNKS[ci]
                nc.tensor.matmul(
                    ovt[h, ci][:, 0:cw], vsb[b, kt][:, h, :],
                    esb[h, kt][:, c0w:c0w + cw],
                    start=(nv[h, ci] == 0), stop=(nv[h, ci] == NT - 1))
                nv[h, ci] += 1

            for kt in range(NT):
                # scores first: all four are flags-0x1 drains; h0's chunks
                # (rows 0-63) run concurrently with h1's (rows 64-127) on
                # disjoint PE row halves -- the hardware-proven pairing class
                scp = {}
                for h in (h0, h1):
                    r0 = (h % 2) * 64
                    scp[h] = sc_psum.tile([TP, N], f32, tag="sc",
                                          name=f"sc{h}_{b}_{kt}")
                    for (c0w, cw) in CHUNKS:
                        nc.tensor.matmul(
                            scp[h][:, c0w:c0w + cw],
                            qkT[b, 4 + h // 2][r0:r0 + 64,
                                               kt * TP:(kt + 1) * TP],
                            qkT[b, h // 2][r0:r0 + 64, c0w:c0w + cw],
                            start=True, stop=False)
                # bias id-matmuls accumulate on top and close each group,
                # then one exp per stream over the whole [112,784] tile
                for h in (h0, h1):
                    for (c0w, cw) in CHUNKS:
                        nc.tensor.matmul(
                            scp[h][:, c0w:c0w + cw], ident_sb[0:TP, 0:TP],
                            bias_sb[h, kt][:, c0w:c0w + cw],
                            start=False, stop=True)
                    et = e_pool.tile([TP, N], bf16, tag="e",
                                     name=f"e{h}_{b}_{kt}")
                    nc.scalar.activation(et, scp[h], EXP)
                    esb[h, kt] = et
                if kt > 0:
                    for h in (h0, h1):
                        for ci in range(len(CHUNKS)):
                            vmm(h, ci, kt - 1)
            for h in (h0, h1):
                for ci in range(len(CHUNKS)):
                    vmm(h, ci, NT - 1)
            # normalize: rows 0..63 * (1 / row 64); recip must not read PSUM
            for h in (h0, h1):
                r0 = (h % 2) * 64
                for ci, (c0w, cw) in enumerate(CHUNKS):
                    srow = r_pool.tile([1, 512], f32, tag="srow", name="srow")
                    nc.vector.tensor_copy(
                        srow[:, 0:cw], ovt[h, ci][KD:KD + 1, 0:cw])
                    rrow = r_pool.tile([1, 512], f32, tag="rrow", name="rrow")
                    nc.vector.reciprocal_approx_fast(
                        rrow[:, 0:cw], srow[:, 0:cw])
                    rb = r_pool.tile([64, 512], f32, tag="rb", name="rb")
                    nc.gpsimd.partition_broadcast(
                        rb[:, 0:cw], rrow[:, 0:cw])
                    nc.vector.tensor_tensor(
                        attnT[b, h // 2][r0:r0 + 64, c0w:c0w + cw],
                        ovt[h, ci][0:KD, 0:cw], rb[:, 0:cw],
                        AluOpType.mult)

        # phase 1: minimal b0 inputs, then pair-0/b0 attention starts while
        # b1's early qkv is emitted
        xts = {}
        with nc.named_scope("qkv_early_b0"):
            xts[0] = emit_xt(0)
            emit_qk_tile(0, 0, xts[0])
            emit_qk_tile(0, 4, xts[0])
            emit_v(0, xts[0])
        bs0 = bias_dma(0)
        with nc.named_scope("headpair0_b0"):
            attention_group(0, 0, bs0)
        with nc.named_scope("qkv_early_b1"):
            xts[1] = emit_xt(1)
            emit_qk_tile(1, 0, xts[1])
            emit_qk_tile(1, 4, xts[1])
            emit_v(1, xts[1])
        with nc.named_scope("headpair0_b1"):
            attention_group(0, 1, bs0)
        # phase 3: remaining q/k feature tiles
        for b in range(BPC):
            with nc.named_scope(f"qkv_rest_b{b}"):
                for ft in (1, 5, 2, 6, 3, 7):
                    emit_qk_tile(b, ft, xts[b])
        # phase 4: remaining head pairs
        for hp in range(1, HEADS // 2):
            bs = bias_dma(hp)
            with nc.named_scope(f"headpair{hp}"):
                attention_group(hp, 0, bs)
                attention_group(hp, 1, bs)

        # ---- output projection ----
        for b in range(BPC):
            with nc.named_scope(f"proj_b{b}"):
                for t in range(NT):
                    ps = pj_psum.tile([TP, 512], f32, tag="op", name=f"po{b}_{t}")
                    for fi in range(CT):
                        nc.tensor.matmul(
                            ps, attnT[b, fi][:, t * TP:(t + 1) * TP], wout_sb[fi],
                            start=(fi == 0), stop=(fi == CT - 1))
                    osb = osb_pool.tile([TP, C], f32, tag="osb", name="osb")
                    nc.vector.tensor_copy(osb, ps)
                    nc.sync.dma_start(out_d[b, t * TP:(t + 1) * TP, :], osb)

    nc.compile()
    _CACHE['nc'] = nc
    return nc


def host_prep(x, w_qkv, pos_table, w_out):
    x = np.asarray(x, np.float32).reshape(B, N, C)
    wq = np.array(np.asarray(w_qkv, np.float32), copy=True)
    wq[:, :C] *= np.float32(1.0 / np.sqrt(KD))
    wq_bf = wq.astype(ml_dtypes.bfloat16)
    idx = _rel_index()
    # bias transposed to [h, k, q]: log-domain (added to scores pre-exp) when
    # IDMM_BIAS, exp-domain (multiplies exp(scores)) otherwise
    bT = np.asarray(pos_table, np.float32)[:, idx].transpose(0, 2, 1)
    if not IDMM_BIAS:
        bT = np.exp(bT)
    biasT = np.ascontiguousarray(bT).astype(ml_dtypes.bfloat16)
    wout = np.ascontiguousarray(np.asarray(w_out, np.float32)).astype(
        ml_dtypes.bfloat16)
    ident = np.eye(128, dtype=ml_dtypes.bfloat16)
    in_maps = []
    for c in range(NCORES):
        xT = np.ascontiguousarray(
            x[c * BPC:(c + 1) * BPC].transpose(0, 2, 1)).astype(
                ml_dtypes.bfloat16)  # [2, 512, 784]
        in_maps.append({"xT": xT, "wqkv": wq_bf, "wout": wout,
                        "biasT": biasT, "ident": ident})
    return in_maps


def run(in_maps, trace=False, trace_cores=None):
    import concourse.bass_utils as bass_utils
    nc = build_nc()
    return bass_utils.run_bass_kernel_spmd(
        nc, in_maps, core_ids=list(range(NCORES)),
        trace=trace, trace_cores=trace_cores)


def kernel(x, w_qkv, pos_table, w_out):
    in_maps = host_prep(x, w_qkv, pos_table, w_out)
    res = run(in_maps)
    out = np.stack([r["out"] for r in res.results])    # [8, 2, 784, 512]
    return np.ascontiguousarray(out.reshape(B, HH, WW, C)).astype(np.float32)
